# revision 59
# baseline (speedup 1.0000x reference)
"""Multi-head attention (B=4, L=2048, D=512, H=8) on 8 Trainium2 NeuronCores.

Sharding: core = (batch b, head-group hg) -> each core handles 1 batch and 4
heads (tensor-parallel column-shard of Wq/Wk/Wv, row-shard of Wo). The two
head-group partial outputs per batch are summed on the host (the TP
all-reduce step of the gather).

Engine plan (~1.5x over the f32r baseline; measured 144-147us vs 217us):
  - All DMA'd operands are bf16 (halves HBM traffic); PE matmuls run bf16
    (same 1 cycle/column as f32r) with f32 PSUM accumulation.
  - kh is stored per-head with the other head's 64 contraction rows zeroed
    so scores run as full 128-row matmuls: the whole kernel stays in the
    128x128 PE tiling mode. (Row-tiled K=64 pairs DO run 2x-concurrent in
    a pure 64-mode stream, but any interleaved 128-mode matmul -- the ctx
    matmuls here -- forces a tiling-mode drain that destroys the overlap;
    measured via probe.)
  - The softmax exp is split across engines to break the ScalarE wall:
    head 0 uses the exact ScalarE exp; head 1 uses a one-instruction
    VectorE Schraudolph exp pt = bitcast_bf16(int16(A16*s + B16)),
    A16 = 128/ln2, B16 tuned for zero-mean error. The softmax
    renormalization cancels the common-mode approximation error; the
    residual ~1.8% per-key noise averages out in the context sum
    (end-to-end rel err 1.41e-2 vs the 2e-2 gate).
  - Head 1's scores use two independent [128,512] psum tiles so each is
    WAR-freed by its own Schraudolph half (PSUM WAR tracking is per-tile).
  - Each phase's normalize (ScalarE sums-row copy -> DVE fast reciprocal ->
    GpSimd partition broadcast -> DVE multiply) is deferred into the next
    phase's first iteration so the cross-engine chain overlaps real work.
    partition_broadcast and the custom-DVE recip read PHYSICAL partition 0
    only (AP base partition is ignored/rejected), hence the ScalarE copy.
  - Dependency-free junk LDWEIGHTS keep the PE array active through exp
    waits and phase boundaries so the HAM clock gate holds 2.4 GHz.
  - The q2=0 output projection is interleaved mid-attention; q2=1 runs as
    a dense PE tail. Projections drain PSUM->SBUF on ScalarE (idle then).
  - Host-side key compaction (masked keys dropped) as in the baseline.
"""
import os
import sys

import numpy as np

for _p in ("/opt/trn_rl_repo", "/root/.axon_site/_ro/trn_rl_repo"):
    if os.path.isdir(_p) and _p not in sys.path:
        sys.path.insert(0, _p)

B, L, D, H = 4, 2048, 512, 8
DK = D // H          # 64
HPG = 4              # heads per group
GD = HPG * DK        # 256
P = 128
NLB = L // 512       # 4 l-blocks of 512
NLC = L // P         # 16 l chunks

A16 = 128.0 / np.log(2.0)    # Schraudolph bf16 scale
B16 = 16248.5                # zero-mean bias (tuned in simulation)
NJUNK = 1                    # junk LDWEIGHTS per iteration (HAM heater)

_CACHE: dict = {}
_RUN_OPTS: dict = {"trace": False}


def _build_nc(ndc: int, nkc: int):
    """Build + compile the Bass program.

    ndc: 4 normally, 5 when q/k/v biases are nonzero (extra contraction chunk
    carrying a ones row x bias row).
    nkc: number of 128-key chunks after host-side compaction of masked keys.
    """
    from contextlib import ExitStack

    import concourse.bacc as bacc
    import concourse.tile as tile
    from concourse import mybir

    f32 = mybir.dt.float32
    f32r = mybir.dt.float32r
    bf16 = mybir.dt.bfloat16
    i16 = mybir.dt.int16
    EXP = mybir.ActivationFunctionType.Exp
    MULT = mybir.AluOpType.mult
    ADD = mybir.AluOpType.add

    nc = bacc.Bacc("TRN2", target_bir_lowering=False, debug=False, num_devices=8)

    NKP = nkc * P
    qT = nc.dram_tensor("qT", [ndc, P, L], bf16, kind="ExternalInput").ap()
    kT = nc.dram_tensor("kT", [ndc, P, NKP], bf16, kind="ExternalInput").ap()
    vT = nc.dram_tensor("vT", [ndc, P, NKP], bf16, kind="ExternalInput").ap()
    wqT = nc.dram_tensor("wqT", [P, ndc, GD], bf16, kind="ExternalInput").ap()
    wkT = nc.dram_tensor("wkT", [P, ndc, GD], bf16, kind="ExternalInput").ap()
    wvT = nc.dram_tensor("wvT", [P, ndc, HPG * 65], bf16, kind="ExternalInput").ap()
    woT = nc.dram_tensor("woT", [P, 2, D], bf16, kind="ExternalInput").ap()
    maskp = nc.dram_tensor("maskp", [P, nkc], f32, kind="ExternalInput").ap()
    o = nc.dram_tensor("o", [NLC, P, D], f32, kind="ExternalOutput").ap()

    with ExitStack() as ctx:
        tc = ctx.enter_context(tile.TileContext(nc))
        const = ctx.enter_context(tc.tile_pool(name="const", bufs=1))
        persist = ctx.enter_context(tc.tile_pool(name="persist", bufs=1))

        wq_sb = const.tile([P, ndc, GD], bf16)
        wk_sb = const.tile([P, ndc, GD], bf16)
        wv_sb = const.tile([P, ndc, HPG * 65], bf16)
        wo_sb = const.tile([P, 2, D], bf16)
        maskp_sb = const.tile([P, nkc], f32)
        dummy_sb = const.tile([1, 8], f32)
        junk = const.tile([P, 512], bf16)
        nc.vector.memset(junk, 0.0)
        nc.sync.dma_start(wk_sb, wkT)
        # preload the exp table set early (overlaps the projection phase)
        nc.vector.memset(dummy_sb, 0.0)
        nc.scalar.activation(dummy_sb, dummy_sb, EXP)

        # persistent activations. kh is stored per-head with the other head's
        # 64 contraction rows zeroed: scores then run as full 128-row matmuls
        # (the pad rows contribute 0), so the whole kernel stays in 128x128
        # PE mode -- tiling-mode switches drain the array.
        qh_sb = [persist.tile([P, L], bf16, name=f"qh{i}") for i in range(2)]
        khp_sb = [[persist.tile([P, NKP], bf16, name=f"khp{i}{j}")
                   for j in range(2)] for i in range(2)]
        vh_sb = persist.tile([P, nkc, HPG, 65], bf16, name="vh")
        ctx_sb = [persist.tile([P, L], bf16, name=f"ctx{i}") for i in range(2)]

        # ---------------- projections ----------------
        with tc.tile_pool(name="xT", bufs=ndc) as xpool, \
             tc.tile_pool(name="ppsum", bufs=6, space="PSUM") as ppsum:
            # HAM warm-up while the first input DMAs are in flight
            warm = ppsum.tile([P, 512], f32, tag="pp", name="warm")
            for _ in range(24):
                nc.tensor.matmul(warm[:, 0:256], lhsT=junk[:, 0:P],
                                 rhs=junk[:, 0:256], start=True, stop=True)
            # dependency-free weight loads bridge the input-DMA wait so the
            # HAM clock gate stays at 2.4 GHz into the projection matmuls
            for _ in range(20):
                nc.tensor.ldweights(junk[:, 0:P])
            kxt = [xpool.tile([P, NKP], bf16, tag="xk", name=f"kxt{dc}")
                   for dc in range(ndc)]
            for dc in range(ndc):
                nc.sync.dma_start(kxt[dc], kT[dc])
            nc.sync.dma_start(wq_sb, wqT)
            qxt = [xpool.tile([P, L], bf16, tag="xq", name=f"qxt{dc}")
                   for dc in range(ndc)]
            for dc in range(ndc):
                nc.sync.dma_start(qxt[dc], qT[dc])
            nc.sync.dma_start(wv_sb, wvT)
            nc.sync.dma_start(maskp_sb, maskp)
            vxt = [xpool.tile([P, NKP], bf16, tag="xv", name=f"vxt{dc}")
                   for dc in range(ndc)]
            for dc in range(ndc):
                nc.sync.dma_start(vxt[dc], vT[dc])
            nc.sync.dma_start(wo_sb, woT)
            # prewarm the GpSimd custom-op library after the input DMAs are
            # queued (first partition_broadcast otherwise pays a ~7us
            # MODIFY_POOL_CONFIG IRAM load mid-attention)
            dummy2_sb = const.tile([1, 8], f32)
            nc.gpsimd.partition_broadcast(dummy2_sb, dummy_sb)
            NKB = (NKP + 511) // 512
            kps = {}
            for dc in range(ndc):
                for hp in range(2):
                    for lb in range(NKB):
                        nb = min(512, NKP - lb * 512)
                        if dc == 0:
                            kps[hp, lb] = ppsum.tile([P, 512], f32, tag="pp",
                                                     name="ps_k")
                        nc.tensor.matmul(
                            kps[hp, lb][:, 0:nb],
                            lhsT=wk_sb[:, dc, hp * P:(hp + 1) * P],
                            rhs=kxt[dc][:, lb * 512:lb * 512 + nb],
                            start=(dc == 0),
                            stop=(dc == ndc - 1),
                        )
            for hp in range(2):
                for hi in range(2):
                    nc.vector.memset(khp_sb[hp][hi], 0.0)
            for (hp, lb), ps in kps.items():
                nb = min(512, NKP - lb * 512)
                for hi in range(2):
                    hb = hi * DK
                    nc.scalar.copy(
                        khp_sb[hp][hi][hb:hb + DK, lb * 512:lb * 512 + nb],
                        ps[hb:hb + DK, 0:nb])
            for hp in range(2):
                for lb in range(NLB):
                    ps = ppsum.tile([P, 512], f32, tag="pp", name="ps_q")
                    for dc in range(ndc):
                        nc.tensor.matmul(
                            ps,
                            lhsT=wq_sb[:, dc, hp * P:(hp + 1) * P],
                            rhs=qxt[dc][:, lb * 512:(lb + 1) * 512],
                            start=(dc == 0),
                            stop=(dc == ndc - 1),
                        )
                    nc.scalar.copy(qh_sb[hp][:, lb * 512:(lb + 1) * 512], ps)
            # V projection: vh[l, :] with mask fold (keys on partitions)
            for lc in range(nkc):
                ps = ppsum.tile([P, HPG * 65], f32, tag="pp", name="ps_v")
                for dc in range(ndc):
                    nc.tensor.matmul(
                        ps,
                        lhsT=vxt[dc][:, lc * P:(lc + 1) * P],
                        rhs=wv_sb[:, dc, :],
                        start=(dc == 0),
                        stop=(dc == ndc - 1),
                    )
                nc.vector.tensor_scalar_mul(
                    vh_sb[:, lc, :, :], ps.rearrange("p (h d) -> p h d", h=HPG),
                    maskp_sb[:, lc:lc + 1],
                )
                # ones-column -> 0/1 mask column (weights there are zero)
                nc.vector.tensor_copy(
                    vh_sb[:, lc, :, DK:DK + 1],
                    maskp_sb[:, lc:lc + 1, None].to_broadcast((P, HPG, 1)),
                )

        # ---------------- attention ----------------
        with tc.tile_pool(name="spsum", bufs=2, space="PSUM") as s_pool, \
             tc.tile_pool(name="cpsum", bufs=2, space="PSUM") as ctx_pool, \
             tc.tile_pool(name="pt", bufs=6) as pt_pool, \
             tc.tile_pool(name="nrm", bufs=4) as nrm_pool, \
             tc.tile_pool(name="osb", bufs=4) as o_pool:
            def emit_oproj(q2, lcs=None):
                # output projection for a finished q half; borrows the scores
                # psum pool. Drains alternate ScalarE/VectorE, DMA per chunk.
                for lc in (lcs if lcs is not None
                           else range(q2 * 8, q2 * 8 + 8)):
                    r = lc % 3
                    if r == 0:
                        ps = s_pool.tile([P, 1024], f32, tag="s", bufs=1,
                                         name="ps_o")[:, 0:D]
                    else:
                        ps = s_pool.tile([P, 512], f32, tag=f"s1{r - 1}",
                                         bufs=1, name="ps_o")
                    for c2 in range(2):
                        nc.tensor.matmul(
                            ps,
                            lhsT=ctx_sb[c2][:, lc * P:(lc + 1) * P],
                            rhs=wo_sb[:, c2, :],
                            start=(c2 == 0), stop=(c2 == 1),
                        )
                    ot = o_pool.tile([P, D], f32, tag="o", name="ot")
                    if lc % 2 == 0:
                        nc.scalar.copy(ot, ps)
                    else:
                        nc.vector.tensor_copy(ot, ps)
                    nc.sync.dma_start(o[lc], ot)

            def emit_normalize(ctxp_p, hp_p, q0_p, his=(0, 1)):
                # normalize: ctx_sb = ctx_ps[0:64] * (1/sums) broadcast,
                # stage-interleaved across heads so the cross-engine chain
                # latency (ACT copy -> DVE recip -> GpSimd bcast -> DVE mult)
                # is paid once, not twice
                # ScalarE drains the whole [65,1024] ctx+sums tile to SBUF
                # (ScalarE time depends only on the free dim, so this costs
                # the same as copying just the sums row) -- this frees the
                # ctx psum tile immediately for the next phase. recip on
                # VectorE; broadcast AND multiply on GpSimd (all-SBUF now),
                # keeping the multiply off the busier VectorE.
                srow, rrow, bc = {}, {}, {}
                for hi in his:
                    srow[hi] = nrm_pool.tile([1, 1024], f32, tag="srow",
                                             name="srow")
                    nc.scalar.copy(srow[hi], ctxp_p[hi][64:65, :])
                for hi in his:
                    rrow[hi] = nrm_pool.tile([1, 1024], f32, tag="rrow",
                                             name="rrow")
                    nc.vector.reciprocal_approx_fast(rrow[hi], srow[hi])
                for hi in his:
                    bc[hi] = nrm_pool.tile([DK, 1024], f32, tag="bc",
                                           name="bc_sb")
                    nc.gpsimd.partition_broadcast(bc[hi], rrow[hi])
                for hi in his:
                    hb = hi * DK
                    nc.vector.tensor_tensor(
                        ctx_sb[hp_p][hb:hb + DK, q0_p:q0_p + 1024],
                        ctxp_p[hi][0:DK, :],
                        bc[hi],
                        MULT,
                    )

            pending_norm = None
            for q2 in range(2):           # q halves of 1024
                q0 = q2 * 1024
                for hp in range(2):       # head pairs
                    ctxp = [ctx_pool.tile([65, 1024], f32, tag="ctx",
                                          name=f"ctx{hi}") for hi in range(2)]

                    def emit_ctx(pv, hi, _c=ctxp, _hp=hp):
                        pt_prev, kcp = pv
                        vlhsT = vh_sb[:, kcp, 2 * _hp + hi, :]
                        for j in range(2):
                            nc.tensor.matmul(
                                _c[hi][:, j * 512:(j + 1) * 512],
                                lhsT=vlhsT,
                                rhs=pt_prev[:, j * 512:(j + 1) * 512],
                                start=(kcp == 0), stop=(kcp == nkc - 1),
                            )

                    prev = [None, None]
                    for kc in range(nkc):
                        # h0: one [P,1024] psum tile (ScalarE exp reads it
                        # whole); h1: two independent [P,512] tiles so each
                        # is WAR-freed by its own VectorE Schraudolph half
                        # (PSUM WAR tracking is tile-granular)
                        s0 = s_pool.tile([P, 1024], f32, tag="s", bufs=1,
                                         name="s0")
                        s1 = [s_pool.tile([P, 512], f32, tag=f"s1{j}", bufs=1,
                                          name=f"s1{j}") for j in range(2)]
                        # scores as full-128-contraction matmuls against the
                        # zero-padded per-head kh (no PE mode switches);
                        # head-major order so each exp can start as soon as
                        # its head's two chunks are done
                        for j in range(2):
                            nc.tensor.matmul(
                                s0[:, j * 512:(j + 1) * 512],
                                lhsT=khp_sb[hp][0][:, kc * P:(kc + 1) * P],
                                rhs=qh_sb[hp][:,
                                              q0 + j * 512:q0 + (j + 1) * 512],
                                start=True, stop=True,
                            )
                        for j in range(2):
                            nc.tensor.matmul(
                                s1[j],
                                lhsT=khp_sb[hp][1][:, kc * P:(kc + 1) * P],
                                rhs=qh_sb[hp][:,
                                              q0 + j * 512:q0 + (j + 1) * 512],
                                start=True, stop=True,
                            )
                        if prev[0] is not None:
                            emit_ctx(prev[0], 0)
                            emit_ctx(prev[1], 1)
                        # dependency-free junk weight loads keep the PE array
                        # active through the exp-wait gap so the HAM clock
                        # gate holds 2.4 GHz (idle windows re-throttle it)
                        for _ in range(NJUNK):
                            nc.tensor.ldweights(junk[:, 0:P])
                        # exp split: head 0 exact on ScalarE, head 1
                        # Schraudolph on VectorE (softmax renormalization
                        # cancels the common-mode approximation error)
                        pt0 = pt_pool.tile([P, 1024], bf16, tag="pt0",
                                           name="pt0")
                        nc.scalar.activation(pt0, s0, EXP)
                        if kc % 4 == 3:
                            # ScalarE takes a quarter of head 1's exps too:
                            # VectorE is otherwise the busier engine
                            pt1 = pt_pool.tile([P, 1024], bf16, tag="pt1b",
                                               name="pt1b")
                            for j in range(2):
                                nc.scalar.activation(
                                    pt1[:, j * 512:(j + 1) * 512], s1[j], EXP)
                        else:
                            pt1i = pt_pool.tile([P, 1024], i16, tag="pt1",
                                                name="pt1i")
                            for j in range(2):
                                nc.vector.tensor_scalar(
                                    pt1i[:, j * 512:(j + 1) * 512], s1[j],
                                    float(A16), float(B16), MULT, ADD)
                            pt1 = pt1i.bitcast(bf16)
                        prev = [(pt0, kc), (pt1, kc)]
                        # previous phase's normalize lands here: after this
                        # phase's first scores/exps are queued (so no engine
                        # bubbles at the boundary) but before this phase's
                        # first ctx matmul reuses the ctx psum slots
                        if kc == 0 and pending_norm is not None:
                            emit_normalize(*pending_norm)
                            pending_norm = None
                            for _ in range(4):
                                nc.tensor.ldweights(junk[:, 0:P])
                        if q2 == 1 and hp == 0 and kc == 2:
                            emit_oproj(0)
                    if q2 == 1 and hp == 1:
                        # last phase: per-head normalize immediately after
                        # each head's final ctx so the tail chain overlaps
                        emit_ctx(prev[0], 0)
                        emit_normalize(ctxp, hp, q0, his=(0,))
                        emit_ctx(prev[1], 1)
                        emit_normalize(ctxp, hp, q0, his=(1,))
                    else:
                        for hi in range(2):
                            emit_ctx(prev[hi], hi)
                        pending_norm = (ctxp, hp, q0)
            emit_oproj(1)

    nc.compile()
    return nc


def _get_nc(ndc: int, nkc: int):
    key = ("nc", ndc, nkc)
    if key not in _CACHE:
        _CACHE[key] = _build_nc(ndc, nkc)
    return _CACHE[key]


def _prep_core(core, q, k, v, masks, wq_w, wq_b, wk_w, wk_b, wv_w, wv_b, ndc,
               nkc):
    import ml_dtypes

    bf16 = ml_dtypes.bfloat16
    b, hg = core // 2, core % 2
    rows = slice(hg * GD, (hg + 1) * GD)
    scale = np.float32(1.0 / np.sqrt(DK))
    NKP = nkc * P
    idx = np.nonzero(masks[b])[0]          # unmasked key positions

    def xt_chunks(x, compact):
        w = NKP if compact else L
        xt = np.zeros((ndc, P, w), np.float32)
        xs = x[idx] if compact else x      # [nk or L, 512]
        xt[:4, :, :xs.shape[0]] = np.ascontiguousarray(xs.T).reshape(4, P, -1)
        if ndc == 5:
            xt[4, 0, :xs.shape[0]] = 1.0   # ones row for the bias chunk
        return xt.astype(bf16)

    def w_chunks(wT, bias, width):
        w = np.zeros((ndc * P, width), np.float32)
        w[:D] = wT
        if ndc == 5:
            w[D] = bias
        return np.ascontiguousarray(
            w.reshape(ndc, P, width).transpose(1, 0, 2)).astype(bf16)

    wqT = (wq_w[rows, :].T * scale).astype(np.float32)          # [512, 256]
    wkT = wk_w[rows, :].T.astype(np.float32)
    wvT = np.zeros((D, HPG * 65), np.float32)
    wvb = np.zeros((HPG * 65,), np.float32)
    wvg = wv_w[rows, :]
    for hh in range(HPG):
        wvT[:, hh * 65:hh * 65 + DK] = wvg[hh * DK:(hh + 1) * DK].T
        wvb[hh * 65:hh * 65 + DK] = wv_b[rows][hh * DK:(hh + 1) * DK]
    maskc = np.zeros((NKP,), np.float32)
    maskc[:len(idx)] = 1.0
    return {
        "qT": xt_chunks(q[b], False),
        "kT": xt_chunks(k[b], True),
        "vT": xt_chunks(v[b], True),
        "wqT": w_chunks(wqT, wq_b[rows] * scale, GD),
        "wkT": w_chunks(wkT, wk_b[rows], GD),
        "wvT": w_chunks(wvT, wvb, HPG * 65),
        "maskp": np.ascontiguousarray(
            maskc.reshape(nkc, P).T.astype(np.float32)),
    }


def kernel(q, k, v, masks, wq_w, wq_b, wk_w, wk_b, wv_w, wv_b, wo_w, wo_b):
    import ml_dtypes

    from concourse.bass_utils import run_bass_kernel_spmd

    bf16 = ml_dtypes.bfloat16
    q = np.asarray(q, np.float32)
    k = np.asarray(k, np.float32)
    v = np.asarray(v, np.float32)
    masks_np = np.asarray(masks)
    args = [np.asarray(a, np.float32) for a in
            (wq_w, wq_b, wk_w, wk_b, wv_w, wv_b, wo_w, wo_b)]
    wq_w, wq_b, wk_w, wk_b, wv_w, wv_b, wo_w, wo_b = args

    ndc = 5 if (np.any(wq_b) or np.any(wk_b) or np.any(wv_b)) else 4
    # key compaction: pad the max unmasked-key count to a 128 multiple
    max_nk = max(int(np.count_nonzero(masks_np[b])) for b in range(B))
    nkc = max(1, (max_nk + P - 1) // P)
    nc = _get_nc(ndc, nkc)

    in_maps = []
    for core in range(8):
        m = _prep_core(core, q, k, v, masks_np, wq_w, wq_b, wk_w, wk_b,
                       wv_w, wv_b, ndc, nkc)
        hg = core % 2
        rows = slice(hg * GD, (hg + 1) * GD)
        m["woT"] = np.ascontiguousarray(
            wo_w[:, rows].T.reshape(2, P, D).transpose(1, 0, 2)).astype(bf16)
        in_maps.append(m)

    res = run_bass_kernel_spmd(nc, in_maps, core_ids=list(range(8)),
                               trace=_RUN_OPTS.get("trace", False),
                               tmpdir=_RUN_OPTS.get("tmpdir"))
    _CACHE["last_result"] = res
    outs = res.results

    O = np.zeros((B, L, D), np.float32)
    for b in range(B):
        O[b] = (outs[2 * b]["o"].reshape(L, D)
                + outs[2 * b + 1]["o"].reshape(L, D))
    O += (wv_b @ wo_w.T + wo_b)[None, None, :] if ndc == 4 else wo_b[None, None, :]
    return O
